# revision 18
# baseline (speedup 1.0000x reference)
"""Trainium2 Bass kernel for nn_AttentionHead: single-head attention with a
causal-style mask and softmax over the query axis.

Sharding: pure data-parallel over batch B=8 -> one batch per NeuronCore.
Params are replicated; no collectives.

Per-core math (T=2048, F=C=512, D=O=64), all matmuls on the PE in bf16:
  qT = Wq @ X^T            (D, T)   lhsT = Wq^T chunks, rhs = X^T chunks
  [k;v]T = Wkv @ Z^T       (2D, T)  packed: k rows 0:64, v rows 64:128
  v = (v^T)^T              (T, O)   PE transpose
  scores = kT.T @ qT       (Tz=i, Tx=j), f32 in PSUM, causal mask j<=i added
  e = exp(scores/8)        ScalarE, fused row-sum via accum_out
  vt = v * (1/rowsum)      fold softmax denominator into v
  out[jt] += e[:,jt].T @ vt   natural-layout accumulation, one [128,64]
                              PSUM tile per output row tile (2 banks total)

The output accumulator banks are zeroed once via a K=1 zero-weight matmul
(clears PSUM has_written); all accumulation matmuls then use start=False.

Emission interleaves projection strips with the attention row blocks they
unblock, so DMA/PE/ACT/DVE pipeline across the whole kernel.

Host side only does layout/dtype prep (transpose + bf16 cast of inputs,
weight repacking) - all FLOPs of the module run on device.
"""

import numpy as np

import ml_dtypes

import concourse.bass as bass
import concourse.mybir as mybir
from concourse import bacc
from concourse.bass_utils import run_bass_kernel_spmd
from concourse.masks import make_causal_mask, make_identity
from concourse.tile import TileContext

BF16 = mybir.dt.bfloat16
F32 = mybir.dt.float32
AF = mybir.ActivationFunctionType
ALU = mybir.AluOpType

B, T, F, C, D, O = 8, 2048, 512, 512, 64, 64
NSTRIP = 4          # t-strips of 512 for phase 1
NI = T // 128       # 16 row blocks of 128
MASK_VAL = -1e10

LAST_RESULTS = None  # test harness reads exec_time_ns from here

_NC = None


def build():
    nc = bacc.Bacc()

    xt = nc.declare_dram_parameter("xt", [128, NSTRIP, 4, 512], BF16, isOutput=False)
    zt = nc.declare_dram_parameter("zt", [128, NSTRIP, 4, 512], BF16, isOutput=False)
    wq = nc.declare_dram_parameter("wq", [128, 4, D], BF16, isOutput=False)
    wkv = nc.declare_dram_parameter("wkv", [128, 4, 2 * D], BF16, isOutput=False)
    bq = nc.declare_dram_parameter("bq", [D, 1], F32, isOutput=False)
    bkv = nc.declare_dram_parameter("bkv", [2 * D, 1], F32, isOutput=False)
    out = nc.declare_dram_parameter("out", [T, O], F32, isOutput=True)

    with TileContext(nc) as tc:
        with (
            tc.tile_pool(name="const", bufs=1) as cpool,
            tc.tile_pool(name="big", bufs=1) as bigpool,
            tc.tile_pool(name="small", bufs=24) as spool,
            tc.tile_pool(name="vt", bufs=6) as vtpool,
            tc.tile_pool(name="mm", bufs=3, space="PSUM") as pp,
            tc.tile_pool(name="acc", bufs=1, space="PSUM") as op,
        ):
            # ---- constants (scalar ring, so xt strip 0 owns the SP ring) ----
            wq_sb = cpool.tile([128, 4 * D], BF16, tag="wq")
            nc.scalar.dma_start(out=wq_sb[:], in_=wq[:])
            wkv_sb = cpool.tile([128, 4 * 2 * D], BF16, tag="wkv")
            nc.scalar.dma_start(out=wkv_sb[:], in_=wkv[:])
            bq_sb = cpool.tile([D, 1], F32, tag="bq")
            nc.scalar.dma_start(out=bq_sb[:], in_=bq[:])
            bkv_sb = cpool.tile([2 * D, 1], F32, tag="bkv")
            nc.scalar.dma_start(out=bkv_sb[:], in_=bkv[:])
            ident = cpool.tile([128, 128], BF16, tag="ident")
            make_identity(nc, ident[:])
            maskadd = cpool.tile([128, 128], F32, tag="mask")
            make_causal_mask(nc, maskadd[:], mask_val=MASK_VAL)
            zrow = cpool.tile([1, 512], BF16, tag="zrow")
            nc.gpsimd.memset(zrow[:], 0.0)
            warm = cpool.tile([128, 2], F32, tag="warm")

            # ---- persistent SBUF tensors ----
            xt_sb = bigpool.tile([128, NSTRIP * 2048], BF16, tag="xt")
            zt_sb = bigpool.tile([128, NSTRIP * 2048], BF16, tag="zt")
            qT_sb = bigpool.tile([D, T], BF16, tag="qT")
            kvT_sb = bigpool.tile([128, T], BF16, tag="kvT")
            v_sb = bigpool.tile([128, NI * O], BF16, tag="v")
            e_sb = [
                bigpool.tile([128, T], BF16, tag=f"e{k}", name=f"e{k}")
                for k in range(4)
            ]
            out_nat = bigpool.tile([128, NI * O], F32, tag="outnat")

            # pull the exp table load forward so it overlaps the input DMA
            nc.scalar.activation(warm[:, 0:1], warm[:, 1:2], AF.Exp, scale=0.0)

            # input loads: xt on the SP HWDGE ring, zt on the ACT ring.
            # The SDMA engines round-robin across ALL queued DMAs, so without
            # ordering every strip would land at the same (late) time. Chain
            # strip s+1 behind strip s per tensor so strip 0 lands ~4x sooner
            # and the compute pipeline can start early.
            from concourse.tile_rust import add_dep_helper

            prev_x = prev_z = None
            for s in range(NSTRIP):
                dx = nc.sync.dma_start(
                    out=xt_sb[:, s * 2048 : (s + 1) * 2048], in_=xt[:, s]
                )
                dz = nc.scalar.dma_start(
                    out=zt_sb[:, s * 2048 : (s + 1) * 2048], in_=zt[:, s]
                )
                if prev_x is not None:
                    add_dep_helper(dx.ins, prev_x.ins, sync=True, reason="strip order")
                    add_dep_helper(dz.ins, prev_z.ins, sync=True, reason="strip order")
                prev_x, prev_z = dx, dz

            # natural-layout output accumulator: tile jt lives at cols 64*jt.
            # Zero both banks with a K=1 zero-weight matmul (start=True also
            # clears has_written, so the accumulation below can be start=False
            # regardless of previous PSUM state).
            acc = op.tile([128, NI * O], F32, tag="acc")
            for bank in range(2):
                nc.tensor.matmul(
                    acc[:, 512 * bank : 512 * (bank + 1)],
                    zrow[:, 0:128],
                    zrow[:, 0:512],
                    start=True,
                    stop=True,
                    skip_group_check=True,
                )

            def strip_proj(s):
                P = pp.tile([128, 1024], F32, tag="mm", name=f"P{s}")
                for fc in range(4):
                    nc.tensor.matmul(
                        P[:, 512:1024],
                        wkv_sb[:, fc * 2 * D : (fc + 1) * 2 * D],
                        zt_sb[:, s * 2048 + fc * 512 : s * 2048 + (fc + 1) * 512],
                        start=(fc == 0),
                        stop=(fc == 3),
                    )
                for fc in range(4):
                    nc.tensor.matmul(
                        P[0:D, 0:512],
                        wq_sb[:, fc * D : (fc + 1) * D],
                        xt_sb[:, s * 2048 + fc * 512 : s * 2048 + (fc + 1) * 512],
                        start=(fc == 0),
                        stop=(fc == 3),
                    )
                nc.vector.tensor_scalar_add(
                    kvT_sb[:, s * 512 : (s + 1) * 512], P[:, 512:1024], bkv_sb[:]
                )
                nc.vector.tensor_scalar_add(
                    qT_sb[:, s * 512 : (s + 1) * 512], P[0:D, 0:512], bq_sb[:]
                )

            def strip_vtrans(s):
                # transpose v^T chunk -> natural v tiles (bf16 pass-through);
                # emitted after the group's scores so the V tile sits behind
                # them in the PSUM slot rotation
                V = pp.tile([128, 1024], BF16, tag="mm", name=f"V{s}")
                for st in range(4):
                    nc.tensor.transpose(
                        V[:, st * O : (st + 1) * O],
                        kvT_sb[D:128, s * 512 + st * 128 : s * 512 + (st + 1) * 128],
                        ident[D:128, D:128],
                    )
                nc.vector.tensor_copy(v_sb[:, s * 4 * O : (s + 1) * 4 * O], V[:, 0 : 4 * O])

            def scores_part(i):
                # scores + mask + exp for row block i; returns softmax partials
                W = 128 * (i + 1)
                e = e_sb[i % 4]
                nh = (W + 1023) // 1024
                partials = []
                for h in range(nh):
                    w = min(1024, W - 1024 * h)
                    S = pp.tile([128, 1024], F32, tag="mm", name=f"S{i}_{h}")
                    for j2 in range((w + 511) // 512):
                        N = min(512, w - 512 * j2)
                        nc.tensor.matmul(
                            S[:, 512 * j2 : 512 * j2 + N],
                            kvT_sb[0:D, 128 * i : 128 * (i + 1)],
                            qT_sb[:, 1024 * h + 512 * j2 : 1024 * h + 512 * j2 + N],
                            start=True,
                            stop=True,
                        )
                    if h == nh - 1:
                        nc.vector.tensor_add(
                            S[:, w - 128 : w], S[:, w - 128 : w], maskadd[:]
                        )
                    sp = spool.tile([128, 1], F32, tag="s", name=f"s{i}_{h}")
                    nc.scalar.activation(
                        e[:, 1024 * h : 1024 * h + w],
                        S[:, 0:w],
                        AF.Exp,
                        scale=0.125,
                        accum_out=sp[:],
                    )
                    partials.append(sp)
                return partials

            def finish_block(i, partials):
                e = e_sb[i % 4]
                if len(partials) == 2:
                    stot = spool.tile([128, 1], F32, tag="s", name=f"st{i}")
                    nc.vector.tensor_add(stot[:], partials[0][:], partials[1][:])
                else:
                    stot = partials[0]
                r = spool.tile([128, 1], F32, tag="r", name=f"r{i}")
                nc.vector.reciprocal(r[:], stot[:])
                vt = vtpool.tile([128, O], BF16, tag="vt", name=f"vt{i}")
                nc.vector.tensor_scalar_mul(vt[:], v_sb[:, O * i : O * (i + 1)], r[:])
                for jt in range(i + 1):
                    nc.tensor.matmul(
                        acc[:, O * jt : O * (jt + 1)],
                        e[:, 128 * jt : 128 * (jt + 1)],
                        vt[:],
                        start=False,
                        stop=(i == NI - 1),
                        skip_group_check=True,
                    )

            # per strip group: projections, then the whole group's scores/exp
            # stream (ACT stays dense), then the lazy v transposes, then the
            # softmax finishes + output accumulation
            for s in range(NSTRIP):
                strip_proj(s)
                parts = [scores_part(i) for i in range(4 * s, 4 * s + 4)]
                strip_vtrans(s)
                for k, i in enumerate(range(4 * s, 4 * s + 4)):
                    finish_block(i, parts[k])

            # ---- store ----
            nc.vector.tensor_copy(out_nat[:], acc[:])
            out_view = out.rearrange("(j p) o -> p j o", p=128)
            nc.sync.dma_start(out=out_view, in_=out_nat[:].rearrange("p (j o) -> p j o", o=O))

    nc.finalize()
    return nc


def _prep_t(x):
    # x: (T, F) f32 -> X^T repacked [128, NSTRIP, 4, 512] bf16
    xt = np.ascontiguousarray(x.T)  # (F, T)
    xt = xt.reshape(4, 128, NSTRIP, 512).transpose(1, 2, 0, 3)
    return np.ascontiguousarray(xt).astype(ml_dtypes.bfloat16)


def _prep_w(w):
    # w: (M, 512) f32 -> W^T repacked [128, 4, M] bf16
    wt = np.ascontiguousarray(w.T)  # (512, M)
    wt = wt.reshape(4, 128, -1).transpose(1, 0, 2)
    return np.ascontiguousarray(wt).astype(ml_dtypes.bfloat16)


def kernel(X, Z, W_q, b_q, W_k, b_k, W_v, b_v):
    global _NC, LAST_RESULTS
    X = np.asarray(X, np.float32)
    Z = np.asarray(Z, np.float32)
    if _NC is None:
        _NC = build()
    wq_h = _prep_w(np.asarray(W_q, np.float32))
    wkv_h = _prep_w(np.concatenate([np.asarray(W_k), np.asarray(W_v)], 0).astype(np.float32))
    bq_h = np.asarray(b_q, np.float32)
    bkv_h = np.concatenate([np.asarray(b_k), np.asarray(b_v)], 0).astype(np.float32)
    in_maps = []
    for b in range(B):
        in_maps.append(
            {
                "xt": _prep_t(X[b]),
                "zt": _prep_t(Z[b]),
                "wq": wq_h,
                "wkv": wkv_h,
                "bq": bq_h,
                "bkv": bkv_h,
            }
        )
    res = run_bass_kernel_spmd(_NC, in_maps, core_ids=list(range(B)))
    LAST_RESULTS = res
    return np.stack([res.results[i]["out"] for i in range(B)], 0).astype(np.float32)


# revision 19
# speedup vs baseline: 1.0860x; 1.0860x over previous
"""Trainium2 Bass kernel for nn_AttentionHead: single-head attention with a
causal-style mask and softmax over the query axis.

Sharding: pure data-parallel over batch B=8 -> one batch per NeuronCore.
Params are replicated; no collectives.

Per-core math (T=2048, F=C=512, D=O=64), all matmuls on the PE in bf16:
  qT = Wq @ X^T            (D, T)   lhsT = Wq^T chunks, rhs = X^T chunks
  [k;v]T = Wkv @ Z^T       (2D, T)  packed: k rows 0:64, v rows 64:128
  v = (v^T)^T              (T, O)   PE transpose
  scores = kT.T @ qT       (Tz=i, Tx=j), f32 in PSUM, causal mask j<=i added
  e = exp(scores/8)        ScalarE, fused row-sum via accum_out
  vt = v * (1/rowsum)      fold softmax denominator into v
  out[jt] += e[:,jt].T @ vt   natural-layout accumulation, one [128,64]
                              PSUM tile per output row tile (2 banks total)

The output accumulator banks are zeroed once via a K=1 zero-weight matmul
(clears PSUM has_written); all accumulation matmuls then use start=False.

Emission interleaves projection strips with the attention row blocks they
unblock, so DMA/PE/ACT/DVE pipeline across the whole kernel.

Host side only does layout/dtype prep (transpose + bf16 cast of inputs,
weight repacking) - all FLOPs of the module run on device.
"""

import numpy as np

import ml_dtypes

import concourse.bass as bass
import concourse.mybir as mybir
from concourse import bacc
from concourse.bass_utils import run_bass_kernel_spmd
from concourse.masks import make_causal_mask, make_identity
from concourse.tile import TileContext

BF16 = mybir.dt.bfloat16
F32 = mybir.dt.float32
AF = mybir.ActivationFunctionType
ALU = mybir.AluOpType

B, T, F, C, D, O = 8, 2048, 512, 512, 64, 64
NSTRIP = 4          # t-strips of 512 for phase 1
NI = T // 128       # 16 row blocks of 128
MASK_VAL = -1e10

LAST_RESULTS = None  # test harness reads exec_time_ns from here

_NC = None


def build():
    nc = bacc.Bacc()

    xt = nc.declare_dram_parameter("xt", [128, NSTRIP, 4, 512], BF16, isOutput=False)
    zt = nc.declare_dram_parameter("zt", [128, NSTRIP, 4, 512], BF16, isOutput=False)
    wq = nc.declare_dram_parameter("wq", [128, 4, D], BF16, isOutput=False)
    wkv = nc.declare_dram_parameter("wkv", [128, 4, 2 * D], BF16, isOutput=False)
    bq = nc.declare_dram_parameter("bq", [D, 1], F32, isOutput=False)
    bkv = nc.declare_dram_parameter("bkv", [2 * D, 1], F32, isOutput=False)
    out = nc.declare_dram_parameter("out", [T, O], F32, isOutput=True)

    with TileContext(nc) as tc:
        with (
            tc.tile_pool(name="const", bufs=1) as cpool,
            tc.tile_pool(name="big", bufs=1) as bigpool,
            tc.tile_pool(name="small", bufs=24) as spool,
            tc.tile_pool(name="vt", bufs=6) as vtpool,
            tc.tile_pool(name="mm", bufs=3, space="PSUM") as pp,
            tc.tile_pool(name="acc", bufs=1, space="PSUM") as op,
        ):
            # ---- constants (scalar ring, so xt strip 0 owns the SP ring) ----
            wq_sb = cpool.tile([128, 4 * D], BF16, tag="wq")
            nc.scalar.dma_start(out=wq_sb[:], in_=wq[:])
            wkv_sb = cpool.tile([128, 4 * 2 * D], BF16, tag="wkv")
            nc.scalar.dma_start(out=wkv_sb[:], in_=wkv[:])
            bq_sb = cpool.tile([D, 1], F32, tag="bq")
            nc.scalar.dma_start(out=bq_sb[:], in_=bq[:])
            bkv_sb = cpool.tile([2 * D, 1], F32, tag="bkv")
            nc.scalar.dma_start(out=bkv_sb[:], in_=bkv[:])
            ident = cpool.tile([128, 128], BF16, tag="ident")
            make_identity(nc, ident[:])
            maskadd = cpool.tile([128, 128], F32, tag="mask")
            make_causal_mask(nc, maskadd[:], mask_val=MASK_VAL)
            zrow = cpool.tile([1, 512], BF16, tag="zrow")
            nc.gpsimd.memset(zrow[:], 0.0)
            warm = cpool.tile([128, 2], F32, tag="warm")

            # ---- persistent SBUF tensors ----
            xt_sb = bigpool.tile([128, NSTRIP * 2048], BF16, tag="xt")
            zt_sb = bigpool.tile([128, NSTRIP * 2048], BF16, tag="zt")
            qT_sb = bigpool.tile([D, T], BF16, tag="qT")
            kvT_sb = bigpool.tile([128, T], BF16, tag="kvT")
            v_sb = bigpool.tile([128, NI * O], BF16, tag="v")
            e_sb = [
                bigpool.tile([128, T], BF16, tag=f"e{k}", name=f"e{k}")
                for k in range(4)
            ]
            out_nat = bigpool.tile([128, NI * O], F32, tag="outnat")

            # pull the exp table load forward so it overlaps the input DMA
            nc.scalar.activation(warm[:, 0:1], warm[:, 1:2], AF.Exp, scale=0.0)

            # input loads: xt on the SP HWDGE ring, zt on the ACT ring.
            # The SDMA engines round-robin across ALL queued DMAs, so without
            # ordering every strip would land at the same (late) time. Chain
            # strip s+1 behind strip s per tensor so strip 0 lands ~4x sooner
            # and the compute pipeline can start early.
            from concourse.tile_rust import add_dep_helper

            prev_x = prev_z = None
            for s in range(NSTRIP):
                dx = nc.sync.dma_start(
                    out=xt_sb[:, s * 2048 : (s + 1) * 2048], in_=xt[:, s]
                )
                dz = nc.scalar.dma_start(
                    out=zt_sb[:, s * 2048 : (s + 1) * 2048], in_=zt[:, s]
                )
                if prev_x is not None:
                    add_dep_helper(dx.ins, prev_x.ins, sync=True, reason="strip order")
                    add_dep_helper(dz.ins, prev_z.ins, sync=True, reason="strip order")
                prev_x, prev_z = dx, dz

            # natural-layout output accumulator: tile jt lives at cols 64*jt.
            # Zero both banks with a K=1 zero-weight matmul (start=True also
            # clears has_written, so the accumulation below can be start=False
            # regardless of previous PSUM state).
            acc = op.tile([128, NI * O], F32, tag="acc")
            for bank in range(2):
                nc.tensor.matmul(
                    acc[:, 512 * bank : 512 * (bank + 1)],
                    zrow[:, 0:128],
                    zrow[:, 0:512],
                    start=True,
                    stop=True,
                    skip_group_check=True,
                )

            def strip_proj(s):
                P = pp.tile([128, 1024], F32, tag="mm", name=f"P{s}")
                for fc in range(4):
                    nc.tensor.matmul(
                        P[:, 512:1024],
                        wkv_sb[:, fc * 2 * D : (fc + 1) * 2 * D],
                        zt_sb[:, s * 2048 + fc * 512 : s * 2048 + (fc + 1) * 512],
                        start=(fc == 0),
                        stop=(fc == 3),
                    )
                for fc in range(4):
                    nc.tensor.matmul(
                        P[0:D, 0:512],
                        wq_sb[:, fc * D : (fc + 1) * D],
                        xt_sb[:, s * 2048 + fc * 512 : s * 2048 + (fc + 1) * 512],
                        start=(fc == 0),
                        stop=(fc == 3),
                    )
                nc.vector.tensor_scalar_add(
                    kvT_sb[:, s * 512 : (s + 1) * 512], P[:, 512:1024], bkv_sb[:]
                )
                nc.vector.tensor_scalar_add(
                    qT_sb[:, s * 512 : (s + 1) * 512], P[0:D, 0:512], bq_sb[:]
                )

            def strip_vtrans(s):
                # transpose v^T chunk -> natural v tiles (bf16 pass-through);
                # emitted after the group's scores so the V tile sits behind
                # them in the PSUM slot rotation
                V = pp.tile([128, 1024], BF16, tag="mm", name=f"V{s}")
                for st in range(4):
                    nc.tensor.transpose(
                        V[:, st * O : (st + 1) * O],
                        kvT_sb[D:128, s * 512 + st * 128 : s * 512 + (st + 1) * 128],
                        ident[D:128, D:128],
                    )
                nc.vector.tensor_copy(v_sb[:, s * 4 * O : (s + 1) * 4 * O], V[:, 0 : 4 * O])

            def scores_part(i):
                # scores + mask + exp for row block i; returns softmax partials
                W = 128 * (i + 1)
                e = e_sb[i % 4]
                nh = (W + 1023) // 1024
                partials = []
                for h in range(nh):
                    w = min(1024, W - 1024 * h)
                    S = pp.tile([128, 1024], F32, tag="mm", name=f"S{i}_{h}")
                    for j2 in range((w + 511) // 512):
                        N = min(512, w - 512 * j2)
                        nc.tensor.matmul(
                            S[:, 512 * j2 : 512 * j2 + N],
                            kvT_sb[0:D, 128 * i : 128 * (i + 1)],
                            qT_sb[:, 1024 * h + 512 * j2 : 1024 * h + 512 * j2 + N],
                            start=True,
                            stop=True,
                        )
                    if h == nh - 1:
                        nc.vector.tensor_add(
                            S[:, w - 128 : w], S[:, w - 128 : w], maskadd[:]
                        )
                    sp = spool.tile([128, 1], F32, tag="s", name=f"s{i}_{h}")
                    nc.scalar.activation(
                        e[:, 1024 * h : 1024 * h + w],
                        S[:, 0:w],
                        AF.Exp,
                        scale=0.125,
                        accum_out=sp[:],
                    )
                    partials.append(sp)
                return partials

            def finish_block(i, partials):
                e = e_sb[i % 4]
                if len(partials) == 2:
                    stot = spool.tile([128, 1], F32, tag="s", name=f"st{i}")
                    nc.vector.tensor_add(stot[:], partials[0][:], partials[1][:])
                else:
                    stot = partials[0]
                r = spool.tile([128, 1], F32, tag="r", name=f"r{i}")
                nc.vector.reciprocal(r[:], stot[:])
                vt = vtpool.tile([128, O], BF16, tag="vt", name=f"vt{i}")
                nc.vector.tensor_scalar_mul(vt[:], v_sb[:, O * i : O * (i + 1)], r[:])
                for jt in range(i + 1):
                    nc.tensor.matmul(
                        acc[:, O * jt : O * (jt + 1)],
                        e[:, 128 * jt : 128 * (jt + 1)],
                        vt[:],
                        start=False,
                        stop=(i == NI - 1),
                        skip_group_check=True,
                    )

            # per strip group: the group's scores/exp stream runs dense on ACT;
            # the NEXT strip's projections are emitted two scores-tiles into
            # the group so their PSUM slot frees early and the DMA latency
            # hides under the current group's exps; v transposes trail the
            # scores; softmax finishes + output accumulation come last
            strip_proj(0)
            for s in range(NSTRIP):
                b = 4 * s
                parts = [scores_part(b), scores_part(b + 1)]
                if s + 1 < NSTRIP:
                    strip_proj(s + 1)
                parts += [scores_part(b + 2), scores_part(b + 3)]
                strip_vtrans(s)
                for k in range(4):
                    finish_block(b + k, parts[k])

            # ---- store ----
            nc.vector.tensor_copy(out_nat[:], acc[:])
            out_view = out.rearrange("(j p) o -> p j o", p=128)
            nc.sync.dma_start(out=out_view, in_=out_nat[:].rearrange("p (j o) -> p j o", o=O))

    nc.finalize()
    return nc


def _prep_t(x):
    # x: (T, F) f32 -> X^T repacked [128, NSTRIP, 4, 512] bf16
    xt = np.ascontiguousarray(x.T)  # (F, T)
    xt = xt.reshape(4, 128, NSTRIP, 512).transpose(1, 2, 0, 3)
    return np.ascontiguousarray(xt).astype(ml_dtypes.bfloat16)


def _prep_w(w):
    # w: (M, 512) f32 -> W^T repacked [128, 4, M] bf16
    wt = np.ascontiguousarray(w.T)  # (512, M)
    wt = wt.reshape(4, 128, -1).transpose(1, 0, 2)
    return np.ascontiguousarray(wt).astype(ml_dtypes.bfloat16)


def kernel(X, Z, W_q, b_q, W_k, b_k, W_v, b_v):
    global _NC, LAST_RESULTS
    X = np.asarray(X, np.float32)
    Z = np.asarray(Z, np.float32)
    if _NC is None:
        _NC = build()
    wq_h = _prep_w(np.asarray(W_q, np.float32))
    wkv_h = _prep_w(np.concatenate([np.asarray(W_k), np.asarray(W_v)], 0).astype(np.float32))
    bq_h = np.asarray(b_q, np.float32)
    bkv_h = np.concatenate([np.asarray(b_k), np.asarray(b_v)], 0).astype(np.float32)
    in_maps = []
    for b in range(B):
        in_maps.append(
            {
                "xt": _prep_t(X[b]),
                "zt": _prep_t(Z[b]),
                "wq": wq_h,
                "wkv": wkv_h,
                "bq": bq_h,
                "bkv": bkv_h,
            }
        )
    res = run_bass_kernel_spmd(_NC, in_maps, core_ids=list(range(B)))
    LAST_RESULTS = res
    return np.stack([res.results[i]["out"] for i in range(B)], 0).astype(np.float32)
